# revision 3
# baseline (speedup 1.0000x reference)
"""Trainium2 Bass kernel for nn_DifferentiableHistogram.

reference:
    px      = x.transpose(0,2,3,1).reshape(B, N, 3)           # B=4, N=65536
    dist_sq = ||px - c||^2  for K=512 bin centers             # (B,N,K)
    w       = exp(-dist_sq / (2*0.02^2))                      # = exp(-1250*d^2)
    w       = w / (sum_k w + 1e-8)
    hist    = sum_n w;  hist = hist / (sum_k hist + 1e-8)     # (B,K)

Sharding: 8 cores = 4 batches x 2 image-row halves. Each core computes a
partial unnormalized histogram over its 32768 pixels (per-pixel softmax
weights are independent across pixels); host sums the two halves per batch
and applies the final K-normalization.

Two device programs:
  * separable — when bin_centers is a 8x8x8 meshgrid (the reference's
    setup_inputs), exp factorizes per axis: w = wx_i * wy_j * wz_l. Only
    24 exps/pixel instead of 512, and the histogram becomes a 3-way
    contraction done on the tensor engine.
  * dense — any bin_centers: dist^2 via matmul (contract over the
    4-vector [x,y,z,1]), exp+row-sum fused on the scalar engine
    (accum_out), per-pixel 1/(S+1e-8) on vector, histogram accumulation
    as a second matmul contracting over pixels.
"""

import math

import numpy as np

import concourse.bass as bass
import concourse.tile as tile
from concourse import bacc, mybir
from concourse.bass_utils import run_bass_kernel_spmd

F32 = mybir.dt.float32
BF16 = mybir.dt.bfloat16
AF = mybir.ActivationFunctionType
ALU = mybir.AluOpType

B, C, H, W = 4, 3, 256, 256
BINS = 8
K = BINS**3
SIGMA = 0.02
LAM = 1.0 / (2.0 * SIGMA**2)  # 1250.0
N_CORES = 8
HH = H // 2          # rows per core: 128
NPIX = HH * W        # pixels per core: 32768
NT = W               # pixel tiles per core (one image column each): 256


def _make_nc():
    return bacc.Bacc("TRN2", target_bir_lowering=False, debug=False)


# --------------------------------------------------------------------------
# separable program: bin_centers = meshgrid(ax, ay, az), K = 8*8*8
# --------------------------------------------------------------------------
def _build_separable(ax, ay, az):
    nc = _make_nc()
    img = nc.dram_tensor("img", [C, HH, W], F32, kind="ExternalInput")
    hist_out = nc.dram_tensor("hist", [BINS, BINS * BINS], F32, kind="ExternalOutput")

    s = math.sqrt(LAM)
    axes = [ax, ay, az]

    with tile.TileContext(nc) as tc:
        with (
            tc.tile_pool(name="const", bufs=1) as const,
            tc.tile_pool(name="work", bufs=1) as work,
            tc.tile_pool(name="psum", bufs=1, space="PSUM") as psum,
        ):
            # load the three channels in image layout (rows x cols)
            X = []
            for c in range(C):
                xc = const.tile([HH, W], F32, tag=f"x{c}")
                nc.sync.dma_start(xc[:], img.ap()[c])
                X.append(xc)

            # per-partition bias columns holding -s*a_i per (channel, bin)
            cb = const.tile([HH, C * BINS], F32, tag="cb")
            for c in range(C):
                for i in range(BINS):
                    nc.gpsimd.memset(
                        cb[:, c * BINS + i : c * BINS + i + 1],
                        -s * float(axes[c][i]),
                    )

            # q[c,i] = 1250*(x - a_i)^2 via ACT Square(s*x - s*a_i)
            # then A[c] = exp(-q) over all 8 i at once, in bf16
            A = []
            for c in range(C):
                t = work.tile([HH, BINS, W], F32, tag=f"t{c}")
                for i in range(BINS):
                    nc.scalar.activation(
                        t[:, i, :], X[c][:], AF.Square,
                        bias=cb[:, c * BINS + i : c * BINS + i + 1], scale=s,
                    )
                a = work.tile([HH, BINS, W], BF16, tag=f"a{c}")
                nc.scalar.activation(a[:], t[:], AF.Exp, scale=-1.0)
                A.append(a)

            # per-channel sums over the 8 axis bins (tree of bf16 adds)
            S_ch = []
            for c in range(C):
                a = A[c]
                p01 = work.tile([HH, W], BF16, tag=f"s01_{c}")
                p23 = work.tile([HH, W], BF16, tag=f"s23_{c}")
                p45 = work.tile([HH, W], BF16, tag=f"s45_{c}")
                p67 = work.tile([HH, W], BF16, tag=f"s67_{c}")
                nc.vector.tensor_add(p01[:], a[:, 0, :], a[:, 1, :])
                nc.vector.tensor_add(p23[:], a[:, 2, :], a[:, 3, :])
                nc.vector.tensor_add(p45[:], a[:, 4, :], a[:, 5, :])
                nc.vector.tensor_add(p67[:], a[:, 6, :], a[:, 7, :])
                nc.vector.tensor_add(p01[:], p01[:], p23[:])
                nc.vector.tensor_add(p45[:], p45[:], p67[:])
                sc = work.tile([HH, W], F32, tag=f"s_{c}")
                nc.vector.tensor_add(sc[:], p01[:], p45[:])
                S_ch.append(sc)

            # r = 1/(SX*SY*SZ + 1e-8), in bf16 for the matmul side
            stot = work.tile([HH, W], F32, tag="stot")
            nc.vector.tensor_mul(stot[:], S_ch[0][:], S_ch[1][:])
            nc.vector.tensor_mul(stot[:], stot[:], S_ch[2][:])
            nc.vector.tensor_scalar_add(stot[:], stot[:], 1e-8)
            rr = work.tile([HH, W], F32, tag="rr")
            nc.vector.reciprocal(rr[:], stot[:])
            rb = work.tile([HH, W], BF16, tag="rb")
            nc.vector.tensor_copy(rb[:], rr[:])

            # A2[i] = Ax_i * r  (broadcast r over i via stride-0 AP)
            a2 = work.tile([HH, BINS, W], BF16, tag="a2")
            rb_b = bass.AP(
                tensor=rb.tensor, offset=rb.offset,
                ap=[rb.ap[0], [0, BINS]] + rb.ap[1:],
            )
            nc.vector.tensor_mul(a2[:], A[0][:], rb_b)

            # Bprod[j*8+l] = Ay_j * Az_l  (outer product via stride-0 APs)
            ay_t, az_t = A[1], A[2]
            bprod = work.tile([HH, BINS * BINS, W], BF16, tag="bprod")
            ay_b = bass.AP(
                tensor=ay_t.tensor, offset=ay_t.offset,
                ap=[ay_t.ap[0], [W, BINS], [0, BINS], [1, W]],
            )
            az_b = bass.AP(
                tensor=az_t.tensor, offset=az_t.offset,
                ap=[az_t.ap[0], [0, BINS], [W, BINS], [1, W]],
            )
            nc.vector.tensor_mul(bprod[:], ay_b, az_b)

            # hist[i, jl] = sum_n A2[n,i] * Bprod[n,jl], contract pixels
            # column by column on the tensor engine, accumulating in PSUM
            hp = psum.tile([BINS, BINS * BINS], F32, tag="hp")
            for f in range(W):
                nc.tensor.matmul(
                    hp[:], lhsT=a2[:, :, f], rhs=bprod[:, :, f],
                    start=(f == 0), stop=(f == W - 1),
                )

            hs = work.tile([BINS, BINS * BINS], F32, tag="hs")
            nc.vector.tensor_copy(hs[:], hp[:])
            nc.sync.dma_start(hist_out.ap(), hs[:])

    nc.compile()
    return nc


# --------------------------------------------------------------------------
# dense program: arbitrary bin_centers
# --------------------------------------------------------------------------
def _build_dense():
    nc = _make_nc()
    xs = nc.dram_tensor("xs", [4, NPIX], F32, kind="ExternalInput")
    caug = nc.dram_tensor("caug", [4, K], F32, kind="ExternalInput")
    hist_out = nc.dram_tensor("hist", [1, K], F32, kind="ExternalOutput")

    with tile.TileContext(nc) as tc:
        with (
            tc.tile_pool(name="const", bufs=1) as const,
            tc.tile_pool(name="wp", bufs=3) as wp,
            tc.tile_pool(name="sp", bufs=4) as sp,
            tc.tile_pool(name="psum", bufs=3, space="PSUM") as psum,
            tc.tile_pool(name="hpsum", bufs=1, space="PSUM") as hpsum,
        ):
            xs_t = const.tile([4, NPIX], F32, tag="xs")
            nc.sync.dma_start(xs_t[:], xs.ap())
            ca = const.tile([4, K], F32, tag="ca")
            nc.sync.dma_start(ca[:], caug.ap())

            # bias = -1250*(x^2+y^2+z^2) in image layout (128 rows, 256 cols)
            X = []
            for c in range(C):
                xc = const.tile([HH, W], F32, tag=f"im{c}")
                nc.sync.dma_start(
                    xc[:], xs.ap()[c].rearrange("(p f) -> p f", p=HH)
                )
                X.append(xc)
            p2 = const.tile([HH, W], F32, tag="p2")
            tmp = const.tile([HH, W], F32, tag="p2tmp")
            nc.vector.scalar_tensor_tensor(
                p2[:], X[0][:], -LAM, X[0][:], ALU.mult, ALU.mult
            )
            nc.vector.scalar_tensor_tensor(
                tmp[:], X[1][:], -LAM, X[1][:], ALU.mult, ALU.mult
            )
            nc.vector.tensor_add(p2[:], p2[:], tmp[:])
            nc.vector.scalar_tensor_tensor(
                tmp[:], X[2][:], -LAM, X[2][:], ALU.mult, ALU.mult
            )
            nc.vector.tensor_add(p2[:], p2[:], tmp[:])

            # pixel tile j = image column j (128 pixels, stride W in xs)
            xsr = xs_t[:].rearrange("c (p f) -> c f p", f=W)

            hp = hpsum.tile([1, K], F32, tag="hp")
            prev = None  # deferred hist matmul for software pipelining
            for j in range(NT):
                u = psum.tile([HH, K], F32, tag="u")
                nc.tensor.matmul(u[:], lhsT=xsr[:, j, :], rhs=ca[:],
                                 start=True, stop=True)
                w = wp.tile([HH, K], BF16, tag="w")
                ssum = sp.tile([HH, 1], F32, tag="ssum")
                nc.scalar.activation(
                    w[:], u[:], AF.Exp,
                    bias=p2[:, j : j + 1], scale=1.0, accum_out=ssum[:],
                )
                rcol = sp.tile([HH, 1], F32, tag="rcol")
                nc.vector.tensor_scalar_add(rcol[:], ssum[:], 1e-8)
                nc.vector.reciprocal(rcol[:], rcol[:])
                rb = sp.tile([HH, 1], BF16, tag="rb")
                nc.vector.tensor_copy(rb[:], rcol[:])
                if prev is not None:
                    pw, prb, pj = prev
                    nc.tensor.matmul(hp[:], lhsT=prb[:], rhs=pw[:],
                                     start=(pj == 0), stop=False)
                prev = (w, rb, j)
            pw, prb, pj = prev
            nc.tensor.matmul(hp[:], lhsT=prb[:], rhs=pw[:],
                             start=False, stop=True)

            hs = sp.tile([1, K], F32, tag="hs")
            nc.vector.tensor_copy(hs[:], hp[:])
            nc.sync.dma_start(hist_out.ap(), hs[:])

    nc.compile()
    return nc


# --------------------------------------------------------------------------
# host orchestration
# --------------------------------------------------------------------------
_PROGRAM_CACHE = {}


def _grid_axes(bin_centers):
    """Return (ax, ay, az) if bin_centers is an ij-ordered meshgrid, else None."""
    if bin_centers.shape != (K, 3):
        return None
    c3 = bin_centers.reshape(BINS, BINS, BINS, 3)
    if (
        np.all(c3[..., 0] == c3[:, :1, :1, 0])
        and np.all(c3[..., 1] == c3[:1, :, :1, 1])
        and np.all(c3[..., 2] == c3[:1, :1, :, 2])
    ):
        return (
            c3[:, 0, 0, 0].astype(np.float64),
            c3[0, :, 0, 1].astype(np.float64),
            c3[0, 0, :, 2].astype(np.float64),
        )
    return None


def kernel(x, bin_centers):
    x = np.asarray(x, dtype=np.float32)
    bin_centers = np.asarray(bin_centers, dtype=np.float32)
    assert x.shape == (B, C, H, W) and bin_centers.shape == (K, 3)

    axes = _grid_axes(bin_centers)

    if axes is not None:
        key = ("sep", bin_centers.tobytes())
        if key not in _PROGRAM_CACHE:
            _PROGRAM_CACHE[key] = _build_separable(*axes)
        nc = _PROGRAM_CACHE[key]
        in_maps = []
        for core in range(N_CORES):
            b, half = divmod(core, 2)
            shard = np.ascontiguousarray(x[b, :, half * HH : (half + 1) * HH, :])
            in_maps.append({"img": shard})
    else:
        key = ("dense",)
        if key not in _PROGRAM_CACHE:
            _PROGRAM_CACHE[key] = _build_dense()
        nc = _PROGRAM_CACHE[key]
        c64 = bin_centers.astype(np.float64)
        caug = np.concatenate(
            [2.0 * LAM * c64.T, -LAM * (c64**2).sum(1)[None]], axis=0
        ).astype(np.float32)
        ones = np.ones((1, NPIX), np.float32)
        in_maps = []
        for core in range(N_CORES):
            b, half = divmod(core, 2)
            shard = x[b, :, half * HH : (half + 1) * HH, :].reshape(C, NPIX)
            in_maps.append(
                {"xs": np.concatenate([shard, ones], 0), "caug": caug}
            )

    global _LAST_RUN
    _LAST_RUN = (nc, in_maps)
    res = run_bass_kernel_spmd(nc, in_maps, core_ids=list(range(N_CORES)))

    parts = [res.results[i]["hist"].reshape(K).astype(np.float32)
             for i in range(N_CORES)]
    hist = np.stack([parts[2 * b] + parts[2 * b + 1] for b in range(B)], 0)
    hist = hist / (hist.sum(axis=1, keepdims=True) + np.float32(1e-8))
    return hist.astype(np.float32)
